# revision 25
# baseline (speedup 1.0000x reference)
"""Discriminative loss kernel for Trainium2 (Bass/Tile), 8-core SPMD.

Data-parallel over batch: core b processes image b (B=8).
Per image the device computes, over P = 512*1024 pixels with D=8 channels
and K=5 instance labels (0 = background):
  pass 1 (flat [128, 4096] pixel layout):
      counts[k] = sum(label==k+1), sums[k,d] = sum_{label==k+1} e_d
      via tensor_scalar(is_equal) + tensor_tensor_reduce; cross-partition
      reduce via a PE ones-matmul.
  tiny device math: centers c = sums/max(counts,1), C2_k = |c_k|^2, and a
      block-diagonal stationary matrix holding -2*c for pass 2.
  pass 2 ((g,d) blocked layout: partition = g*8+d, g=16 pixel groups):
      psum[(g,k),f] = sum_d(-2 c_kd e_d) + |e|^2       (two PE matmuls)
      d = sqrt(psum + C2_k); h = relu(d - 0.5); h2 = h^2   (ACT)
      inst_sum[k] += sum_f h2 * (label==k+1)           (DVE TTR)
Host combines the per-image scalars into the final 4 losses.
"""

import os
import sys

import numpy as np

for _p in ("/opt/trn_rl_repo", "/root/.axon_site/_ro/trn_rl_repo"):
    if os.path.isdir(_p) and _p not in sys.path:
        sys.path.insert(0, _p)

import concourse.bass as bass
import concourse.tile as tile
from concourse import mybir
from concourse.bass_utils import run_bass_kernel_spmd

F32 = mybir.dt.float32
F32R = mybir.dt.float32r
BF16 = mybir.dt.bfloat16
Alu = mybir.AluOpType
Act = mybir.ActivationFunctionType

B, D, H, W = 8, 8, 512, 1024
P = H * W          # 524288 pixels
K = 5
R = 128            # sbuf partitions
COLS = P // R      # 4096
NCH = 8            # pass-1 load/cast chunks
CW = COLS // NCH   # 512
G = 16             # pass-2 pixel groups
GPP = P // G       # 32768 pixels per group
F = 1024           # pass-2 tile width
NT = GPP // F      # 32 tiles
DELTA_V = 0.5
DELTA_D = 3.0
ALPHA, BETA, GAMMA = 1.0, 1.0, 0.001


def _to_bf16(a):
    import ml_dtypes
    return a.astype(ml_dtypes.bfloat16)


def _build_consts():
    sel_cnt = np.zeros((R, 40), np.float32)
    sel_sum = np.zeros((R, 40), np.float32)
    for k in range(K):
        for d in range(D):
            sel_cnt[9 * k + 8, 8 * k + d] = 1.0
            sel_sum[9 * k + d, 8 * k + d] = 1.0
    sum5 = np.zeros((R, K), np.float32)
    for k in range(K):
        for d in range(D):
            sum5[8 * k + d, k] = 1.0
    rep80 = np.zeros((R, 80), np.float32)
    for g in range(G):
        for k in range(K):
            rep80[k, 5 * g + k] = 1.0
    smat = np.zeros((R, 80), np.float32)
    for kk in range(K):
        for d in range(D):
            for g in range(G):
                smat[8 * kk + d, 5 * g + kk] = 1.0
    dsel = np.zeros((R, R), np.float32)
    for k in range(K):
        for d in range(D):
            for g in range(G):
                dsel[8 * k + d, 8 * g + d] = 1.0
    blockmask = np.zeros((R, 80), np.float32)
    for g in range(G):
        for d in range(D):
            for k in range(K):
                blockmask[8 * g + d, 5 * g + k] = 1.0
    ones_col = np.ones((R, 1), np.float32)
    kpat = np.zeros((R, K), np.float32)
    kvec = np.zeros((R, 1), np.float32)
    for g in range(G):
        for k in range(K):
            kpat[5 * g + k, k] = 1.0
            kvec[5 * g + k, 0] = float(k + 1)
    return dict(sel_cnt=sel_cnt, sel_sum=sel_sum, sum5=sum5, rep80=rep80,
                smat=smat, dsel=dsel, blockmask=blockmask, ones_col=ones_col,
                kpat=kpat, kvec=kvec,
                blockmask_bf=_to_bf16(blockmask),
                ones_sq_bf=_to_bf16(np.ones((R, R), np.float32)))


def _ap(handle, offset, dims):
    return bass.AP(tensor=handle.tensor if isinstance(handle, bass.AP) else handle,
                   offset=offset, ap=[list(x) for x in dims])


def _split_multiwait(nc):
    """This container's walrus encodes at most one sync-wait per instruction;
    Tile's tail drain carries one wait per outstanding DMA queue. Hoist the
    extra waits onto single-wait drains inserted just before."""
    n_split = 0
    for blk in nc.m.functions[0].blocks:
        out = []
        changed = False
        for i in blk.instructions:
            si = i.sync_info
            if si is not None and len(si.on_wait) > 1:
                waits = list(si.on_wait)
                for w in waits[:-1]:
                    d = mybir.InstDrain(
                        name=nc.get_next_instruction_name(), ins=[], outs=[])
                    d.engine = i.engine
                    d.sync_info = mybir.SyncInfo(on_wait=[w], on_update=[])
                    out.append(d)
                    n_split += 1
                i.sync_info = mybir.SyncInfo(
                    on_wait=[waits[-1]], on_update=list(si.on_update))
                changed = True
            out.append(i)
        if changed:
            blk.instructions = out
    return n_split


def build_program():
    nc = bass.Bass()
    emb = nc.declare_dram_parameter("emb", [D, P], F32, isOutput=False)
    maskf = nc.declare_dram_parameter("maskf", [P], F32, isOutput=False)
    o_stats = nc.declare_dram_parameter("o_stats", [45], F32, isOutput=True)
    o_c = nc.declare_dram_parameter("o_c", [40], F32, isOutput=True)
    o_inst = nc.declare_dram_parameter("o_inst", [K], F32, isOutput=True)
    labbf = nc.dram_tensor("labbf", [P], BF16)
    ebf = nc.dram_tensor("ebf", [D, P], BF16)

    cn = {k: nc.inline_tensor(v, name=f"c_{k}") for k, v in _build_consts().items()}

    with tile.TileContext(nc) as tc:
        with tc.tile_pool(name="singles", bufs=1) as singles, \
             tc.tile_pool(name="p1", bufs=2) as p1, \
             tc.tile_pool(name="mpool", bufs=2) as mpool, \
             tc.tile_pool(name="p2", bufs=2) as p2, \
             tc.tile_pool(name="psum_s", bufs=2, space="PSUM") as psum_s, \
             tc.tile_pool(name="psumR", bufs=2, space="PSUM") as psumR, \
             tc.tile_pool(name="psum2", bufs=2, space="PSUM") as psum2:

            # load constants
            sb = {}
            for name, h in cn.items():
                t = singles.tile(list(h.shape), h.dtype, tag=f"c_{name}")
                nc.sync.dma_start(out=t, in_=h[:])
                sb[name] = t

            # constants used as activation biases
            for cval in (0.0, -DELTA_V):
                ct = singles.tile([R, 1], F32, tag=f"bias_{cval}")
                nc.vector.memset(ct, cval)
                nc.const_aps.aps[(F32, cval)] = ct[:]

            instacc = singles.tile([R, NT], F32)
            ebf_sb = singles.tile([R, D, COLS], BF16)
            lf_sb = singles.tile([R, COLS], F32)

            # ---------------- pass 1 ----------------
            # stage A: load f32, cast embedding/labels to bf16, stash scratch
            for ch in range(NCH):
                et = p1.tile([R, D, CW], F32, tag="et")
                nc.sync.dma_start(
                    out=et, in_=_ap(emb, ch * CW, [[COLS, R], [P, D], [1, CW]]))
                nc.vector.tensor_scalar(
                    out=ebf_sb[:, :, ch * CW:(ch + 1) * CW], in0=et,
                    scalar1=1.0, scalar2=None, op0=Alu.mult)
                nc.sync.dma_start(
                    out=lf_sb[:, ch * CW:(ch + 1) * CW],
                    in_=_ap(maskf, ch * CW, [[COLS, R], [1, CW]]))
                lb = p1.tile([R, CW], BF16, tag="lb")
                nc.vector.tensor_scalar(
                    out=lb, in0=lf_sb[:, ch * CW:(ch + 1) * CW],
                    scalar1=1.0, scalar2=None, op0=Alu.mult)
                nc.sync.dma_start(
                    out=_ap(labbf, ch * CW, [[COLS, R], [1, CW]]), in_=lb)
                nc.sync.dma_start(
                    out=_ap(ebf, ch * CW, [[COLS, R], [P, D], [1, CW]]),
                    in_=ebf_sb[:, :, ch * CW:(ch + 1) * CW])
            # stage B: per (k, d): bf16 product plane, PE column-sum into
            # psum, ACT copy-with-accumulate -> accB[:, col] (scale 1/128;
            # the later ones-matmul over 128 identical rows multiplies back)
            accB = singles.tile([R, 45], F32)
            NJ = COLS // 512
            for k in range(K):
                mk = mpool.tile([R, COLS], BF16, tag="mk")
                nc.vector.tensor_scalar(
                    out=mk, in0=lf_sb, scalar1=float(k + 1), scalar2=None,
                    op0=Alu.is_equal)
                for d in range(-1, D):
                    if d < 0:
                        plane = mk
                        col = 9 * k + 8
                    else:
                        q = mpool.tile([R, COLS], BF16, tag="q")
                        nc.vector.tensor_tensor(
                            out=q, in0=ebf_sb[:, d, :], in1=mk, op=Alu.mult)
                        plane = q
                        col = 9 * k + d
                    ps = psumR.tile([R, 512], F32, tag="red")
                    for j in range(NJ):
                        nc.tensor.matmul(
                            ps, sb["ones_sq_bf"], plane[:, j * 512:(j + 1) * 512],
                            start=(j == 0), stop=(j == NJ - 1))
                    junkA = mpool.tile([R, 512], F32, tag="junkA")
                    nc.scalar.activation(
                        out=junkA, in_=ps, func=Act.Copy, bias=0.0,
                        scale=1.0 / R, accum_out=accB[:, col:col + 1])

            # cross-partition: 128 identical rows x (stats/128) -> stats
            ps45 = psum_s.tile([45, 1], F32, tag="small")
            nc.tensor.matmul(ps45, accB, sb["ones_col"], start=True, stop=True)
            sb45 = singles.tile([R, 1], F32)
            nc.vector.memset(sb45, 0.0)
            nc.scalar.copy(out=sb45[:45, :], in_=ps45)
            nc.sync.dma_start(out=o_stats[:].unsqueeze(1), in_=sb45[:45, :])

            # ---------------- tiny math: centers ----------------
            ps40a = psum_s.tile([40, 1], F32, tag="small")
            nc.tensor.matmul(ps40a, sb["sel_cnt"], sb45, start=True, stop=True)
            ps40b = psum_s.tile([40, 1], F32, tag="small")
            nc.tensor.matmul(ps40b, sb["sel_sum"], sb45, start=True, stop=True)
            cntc = singles.tile([R, 1], F32)
            nc.vector.memset(cntc, 0.0)
            nc.vector.tensor_scalar(out=cntc[:40, :], in0=ps40a, scalar1=1.0,
                                    scalar2=None, op0=Alu.max)
            inv = singles.tile([R, 1], F32)
            nc.vector.memset(inv, 0.0)
            nc.vector.reciprocal(out=inv[:40, :], in_=cntc[:40, :])
            c40 = singles.tile([R, 1], F32)
            nc.vector.memset(c40, 0.0)
            nc.vector.tensor_tensor(out=c40[:40, :], in0=ps40b, in1=inv[:40, :],
                                    op=Alu.mult)
            nc.sync.dma_start(out=o_c[:].unsqueeze(1), in_=c40[:40, :])
            cm2 = singles.tile([R, 1], F32)
            nc.vector.memset(cm2, 0.0)
            nc.vector.tensor_scalar(out=cm2[:40, :], in0=c40[:40, :],
                                    scalar1=-2.0, scalar2=None, op0=Alu.mult)
            csq = singles.tile([R, 1], F32)
            nc.vector.memset(csq, 0.0)
            nc.vector.tensor_tensor(out=csq[:40, :], in0=c40[:40, :],
                                    in1=c40[:40, :], op=Alu.mult)
            ps5 = psum_s.tile([K, 1], F32, tag="small")
            nc.tensor.matmul(ps5, sb["sum5"], csq, start=True, stop=True)
            c2sb = singles.tile([R, 1], F32)
            nc.vector.memset(c2sb, 0.0)
            nc.scalar.copy(out=c2sb[:K, :], in_=ps5)
            ps80 = psum_s.tile([80, 1], F32, tag="small")
            nc.tensor.matmul(ps80, sb["rep80"], c2sb, start=True, stop=True)
            c2bias = singles.tile([R, 1], F32)
            nc.vector.memset(c2bias, 0.0)
            nc.scalar.copy(out=c2bias[:80, :], in_=ps80)

            # block-diagonal stationary: cblk[8g+d, 5g+k] = -2*c[k,d]
            rhsS = singles.tile([R, 80], F32)
            nc.vector.tensor_scalar(out=rhsS, in0=sb["smat"], scalar1=cm2,
                                    scalar2=None, op0=Alu.mult)
            psD = psum_s.tile([R, 80], F32, tag="small")
            nc.tensor.matmul(psD, sb["dsel"], rhsS, start=True, stop=True)
            cblk = singles.tile([R, 80], F32)
            nc.vector.tensor_tensor(out=cblk, in0=psD, in1=sb["blockmask"],
                                    op=Alu.mult)
            cblk_bf = singles.tile([R, 80], BF16)
            nc.vector.tensor_scalar(out=cblk_bf, in0=cblk, scalar1=1.0,
                                    scalar2=None, op0=Alu.mult)

            # ---------------- pass 2 ----------------
            for t in range(NT):
                et2 = p2.tile([R, F], BF16, tag="et2")
                nc.sync.dma_start(
                    out=et2, in_=_ap(ebf, t * F, [[GPP, G], [P, D], [1, F]]))
                ld5 = p2.tile([80, F], BF16, tag="ld5")
                nc.sync.dma_start(
                    out=ld5, in_=_ap(labbf, t * F, [[GPP, G], [0, K], [1, F]]))
                sq = p2.tile([R, F], BF16, tag="sq")
                if t % 2 == 0:
                    nc.gpsimd.tensor_mul(sq, et2, et2)
                else:
                    nc.scalar.square(sq, et2)
                pt = psum2.tile([80, F], F32, tag="pt")
                for hh_ in range(2):
                    sl = slice(hh_ * 512, (hh_ + 1) * 512)
                    nc.tensor.matmul(pt[:, sl], cblk_bf, et2[:, sl],
                                     start=True, stop=False)
                    nc.tensor.matmul(pt[:, sl], sb["blockmask_bf"], sq[:, sl],
                                     start=False, stop=True)
                dd = p2.tile([80, F], F32, tag="dd")
                nc.scalar.activation(out=dd, in_=pt, func=Act.Sqrt,
                                     bias=c2bias[:80, :], scale=1.0)
                hh = p2.tile([80, F], F32, tag="hh")
                nc.scalar.activation(out=hh, in_=dd, func=Act.Relu,
                                     bias=-DELTA_V, scale=1.0)
                h2 = p2.tile([80, F], F32, tag="h2")
                if t % 2 == 0:
                    nc.scalar.square(h2, hh)
                else:
                    nc.gpsimd.tensor_mul(h2, hh, hh)
                mm = p2.tile([80, F], BF16, tag="mm")
                nc.vector.tensor_scalar(out=mm, in0=ld5,
                                        scalar1=sb["kvec"][:80, :],
                                        scalar2=None, op0=Alu.is_equal)
                junk3 = p2.tile([80, F], F32, tag="junk3")
                nc.vector.affine_mul_reduce(
                    out=junk3, accum_out=instacc[:80, t:t + 1],
                    in0=h2, in1=mm, scale=1.0, bias=0.0)

            inst1 = singles.tile([R, 1], F32)
            nc.vector.memset(inst1, 0.0)
            nc.vector.tensor_reduce(out=inst1[:80, :], in_=instacc[:80, :],
                                    axis=mybir.AxisListType.X, op=Alu.add)
            psI = psum_s.tile([1, K], F32, tag="small")
            nc.tensor.matmul(psI, inst1, sb["kpat"], start=True, stop=True)
            instsb = singles.tile([1, K], F32)
            nc.scalar.copy(out=instsb, in_=psI)
            nc.sync.dma_start(out=o_inst[:].unsqueeze(0), in_=instsb)

    from concourse.library_overlay import lower_extended_insts
    lower_extended_insts(nc)
    _split_multiwait(nc)
    return nc


_NC_CACHE = None


def _get_nc():
    global _NC_CACHE
    if _NC_CACHE is None:
        _NC_CACHE = build_program()
    return _NC_CACHE


def run_device(embedding, maskf, trace=False):
    nc = _get_nc()
    in_maps = [
        {"emb": np.ascontiguousarray(embedding[b].reshape(D, P)),
         "maskf": np.ascontiguousarray(maskf[b].reshape(P))}
        for b in range(B)
    ]
    res = run_bass_kernel_spmd(nc, in_maps, list(range(B)), trace=trace)
    return res


def finalize(per_core):
    """Combine per-image device stats into the 4 reference losses."""
    loss_var_b = np.zeros(B, np.float32)
    loss_dist_b = np.zeros(B, np.float32)
    loss_reg_b = np.zeros(B, np.float32)
    Ns = np.zeros(B, np.float32)
    iu = np.triu(np.ones((K, K), bool), k=1)
    for b in range(B):
        s45 = per_core[b]["o_stats"].astype(np.float32)
        c = per_core[b]["o_c"].astype(np.float32).reshape(K, D)
        inst = per_core[b]["o_inst"].astype(np.float32)
        counts = s45[8::9]
        present = counts > 0
        presentf = present.astype(np.float32)
        N = presentf.sum()
        Ns[b] = N
        inst_mean = inst / np.maximum(counts, 1.0)
        loss_var_b[b] = (inst_mean * presentf).sum() / max(N, 1.0)
        diff = c[:, None, :] - c[None, :, :]
        dist_sq = (diff ** 2).sum(-1)
        pair_mask = present[:, None] & present[None, :] & iu
        safe = np.sqrt(np.where(pair_mask, dist_sq, 1.0))
        term = np.maximum(2.0 * DELTA_D - safe, 0.0) ** 2 * pair_mask
        n_pairs = N * (N - 1.0) / 2.0
        loss_dist_b[b] = term.sum() / (n_pairs if N > 1 else 1.0)
        c_norm = np.sqrt(np.where(present, (c ** 2).sum(-1), 1.0))
        loss_reg_b[b] = (c_norm * presentf).sum() / max(N, 1.0)
    has = (Ns > 0).astype(np.float32)
    denom = max(has.sum(), 1.0)
    loss_var = float((loss_var_b * has).sum() / denom)
    loss_dist = float((loss_dist_b * has).sum() / denom)
    loss_reg = float((loss_reg_b * has).sum() / denom)
    total = ALPHA * loss_var + BETA * loss_dist + GAMMA * loss_reg
    return (np.float32(total), np.float32(loss_var),
            np.float32(loss_dist), np.float32(loss_reg))


def kernel(embedding, instance_mask):
    embedding = np.asarray(embedding, dtype=np.float32)
    maskf = np.asarray(instance_mask).astype(np.float32)
    res = run_device(embedding, maskf, trace=False)
    return finalize(res.results)
